# revision 17
# baseline (speedup 1.0000x reference)
"""Cross-attention kernel for 8 TRN2 NeuronCores (Bass/Tile).

Reference (fp32):
    q = x @ Wq; k = ctx @ Wk; v = ctx @ Wv        (8 heads, d=64)
    sim = q k^T * d^-0.5 ; attn = softmax(sim)
    out = (attn v) @ Wo + bo

Sharding (data-parallel, no FLOP duplication): core c -> batch c//2,
head-group c%2 (4 heads).  Each core computes a partial [2048, 1024]
output; the host sums the two partials per batch and adds bo.

Per-core dataflow:
  - inputs x/ctx and Wq/Wk/Wv ship as fp8e4 (weights pre-scaled x32 on
    the host; fp8 noise enters PRE-softmax where the 1/8 sim scaling
    damps it to ~1e-3, unlike post-softmax fp8 which has no sqrt(N)
    attenuation) -- halves input DMA and doubles projection matmuls
    via DoubleRow (two K-tiles per instruction)
  - QT[d,i] = Wq^T x^T ; KT[d,j] = Wk^T ctx^T ; V[j,d] = ctx Wv, all
    evacuated to bf16; V gets a ones column ([V_h | 1], memset)
  - simT[j,i] = KT_h @ QT_h (K=64 bf16; head pairs on PE row groups),
    values x1024 from the weight scaling
  - expT = exp(sim/(8*1024)) on ScalarE -> bf16 (unsafe softmax)
  - av = [V_h|1]^T @ expT (bf16) -> [65, i] fp32 PSUM; row 64 = denom
  - norm: denom row -> DRAM -> [8,128] transpose read -> reciprocal
    (x 1/32 undoes the Wv scale) -> DRAM -> 0-stride broadcast read ->
    multiply -> o2t bf16
  - out = o2tT^T @ Wo (bf16) -> psum -> bf16 staging -> DRAM; host
    accumulates partials in fp32 and adds bo
The kernel is ScalarE-bound (64 exp ops over 8.4M elements, ~71us);
PE work (~70us) is balanced against it so the HAM clock never sees an
idle Tensor engine mid-schedule.
"""

import numpy as np
import ml_dtypes

import concourse.bass as bass
import concourse.tile as tile
from concourse import bacc, mybir
from concourse.bass_utils import run_bass_kernel_spmd

B = 4
I = 2048
J = 1024
FQ = 1024
FC = 768
DH = 64
HPC = 4
DG = HPC * DH      # 256
E = 1024
P = 128
N_CORES = 8
IH = I // 2        # 1024
WSCALE = 32.0      # host premultiplier on Wq/Wk/Wv (fp8 subnormal dodge)

F32 = mybir.dt.float32
BF16 = mybir.dt.bfloat16
FP8 = mybir.dt.float8e4

KQ = FQ // P       # 8
KC = FC // P       # 6
TD = DG // P       # 2
JBN = J // P       # 8
ICN = 4            # x i-chunks
KCA = 2            # ctx kb's in first half (pair-aligned)
DR = mybir.MatmulPerfMode.DoubleRow


def _build():
    nc = bacc.Bacc()
    xt = nc.declare_dram_parameter("xt", [P, KQ * I], FP8, isOutput=False)
    ctxt = nc.declare_dram_parameter("ctxt", [P, KC * J], FP8, isOutput=False)
    wq = nc.declare_dram_parameter("wq", [P, KQ * DG], FP8, isOutput=False)
    wk = nc.declare_dram_parameter("wk", [P, KC * DG], FP8, isOutput=False)
    wv = nc.declare_dram_parameter("wv", [P, KC * DG], BF16, isOutput=False)
    ctxv = nc.declare_dram_parameter("ctxv", [P, KC * J], BF16,
                                     isOutput=False)
    wo = nc.declare_dram_parameter("wo", [P, TD * E], BF16, isOutput=False)
    out = nc.declare_dram_parameter("out", [I, E], BF16, isOutput=True)
    brc = nc.dram_tensor("brc", [2 * HPC, IH], F32)
    brc2 = nc.dram_tensor("brc2", [2 * HPC, IH], F32)

    with tile.TileContext(nc) as tc:
        with (
            tc.tile_pool(name="consts", bufs=1) as consts,
            tc.tile_pool(name="expp", bufs=36) as expp,
            tc.tile_pool(name="misc", bufs=3) as misc,
            tc.tile_pool(name="outp", bufs=6) as outp,
            tc.tile_pool(name="pp", bufs=2, space="PSUM") as pp,
            tc.tile_pool(name="pp2", bufs=2, space="PSUM") as pp2,
            tc.tile_pool(name="avp", bufs=1, space="PSUM") as avpool,
        ):
            # ---- PE warm-up: junk matmuls trip the HAM clock-gate (cold
            # PE runs 0.65-1.2 GHz) and bridge until x0 lands
            junk = consts.tile([P, P], BF16, tag="junk")
            nc.vector.memset(junk, 0.0)
            jps = pp2.tile([P, 512], F32, tag="pp2", name="jps")
            for w in range(34):
                nc.tensor.matmul(jps[:, :P], lhsT=junk, rhs=junk,
                                 start=True, stop=True)

            # ---- loads, ordered by the critical path to the first exp:
            # wq+x0 (QT is the PE's first real work), x1, ctx halves + wk
            # (KT), then the rest.  All fp8 -> half the bytes.
            wq_sb = consts.tile([P, KQ, DG], FP8, tag="wq_sb")
            nc.sync.dma_start(
                out=wq_sb, in_=wq[:, :].rearrange("p (kb d) -> p kb d", kb=KQ))
            xq_sb = consts.tile([P, ICN, KQ, 512], FP8, tag="xq_sb")

            def load_x(ich):
                nc.sync.dma_start(
                    out=xq_sb[:, ich],
                    in_=xt[:, ich * KQ * 512:(ich + 1) * KQ * 512]
                    .rearrange("p (kb i) -> p kb i", kb=KQ))

            load_x(0)
            load_x(1)
            # ctx split (2,4) kb's: DoubleRow kb-pairs never straddle tiles
            ctx_sb = [consts.tile([P, KCA, J], FP8, tag="ctxA", name="ctxA"),
                      consts.tile([P, KC - KCA, J], FP8, tag="ctxB",
                                  name="ctxB")]
            nc.sync.dma_start(
                out=ctx_sb[0],
                in_=ctxt[:, :KCA * J].rearrange("p (kb j) -> p kb j", kb=KCA))
            wk_sb = consts.tile([P, KC, DG], FP8, tag="wk_sb")
            nc.sync.dma_start(
                out=wk_sb, in_=wk[:, :].rearrange("p (kb d) -> p kb d", kb=KC))
            nc.sync.dma_start(
                out=ctx_sb[1],
                in_=ctxt[:, KCA * J:].rearrange("p (kb j) -> p kb j",
                                                kb=KC - KCA))
            wv_sb = consts.tile([P, KC, DG], BF16, tag="wv_sb")
            nc.sync.dma_start(
                out=wv_sb, in_=wv[:, :].rearrange("p (kb d) -> p kb d", kb=KC))
            ctxv_sb = consts.tile([P, KC, J], BF16, tag="ctxv_sb")
            nc.sync.dma_start(
                out=ctxv_sb,
                in_=ctxv[:, :].rearrange("p (kb j) -> p kb j", kb=KC))
            load_x(2)
            load_x(3)
            wo_sb = consts.tile([P, TD, E], BF16, tag="wo_sb")
            nc.sync.dma_start(
                out=wo_sb, in_=wo[:, :].rearrange("p (kb e) -> p kb e", kb=TD))

            # kb-pair views for DoubleRow: (tile, local pair index)
            def ctx_pair(pr):
                return ctx_sb[0] if pr == 0 else ctx_sb[1], 0 if pr == 0 \
                    else pr - 1

            # ---- projections (fp8 DoubleRow: 2 kb per matmul)
            kt_sb = [consts.tile([P, J], BF16, tag=f"kt{t}", name=f"kt{t}")
                     for t in range(TD)]

            def emit_kt(t):
                for nch in range(2):
                    ps = pp2.tile([P, 512], F32, tag="pp2", name="ktps")
                    for pr in range(KC // 2):
                        cs, lp = ctx_pair(pr)
                        nc.tensor.matmul(
                            ps,
                            lhsT=wk_sb[:, 2 * pr:2 * pr + 2,
                                       t * P:(t + 1) * P],
                            rhs=cs[:, 2 * lp:2 * lp + 2,
                                   nch * 512:(nch + 1) * 512],
                            start=(pr == 0), stop=(pr == KC // 2 - 1),
                            perf_mode=DR,
                        )
                    nc.vector.tensor_copy(
                        kt_sb[t][:, nch * 512:(nch + 1) * 512], ps)

            v_sb = [consts.tile([P, HPC, DH + 1], BF16, tag=f"v{jb}",
                                name=f"v{jb}") for jb in range(JBN)]
            for jb in range(JBN):
                nc.gpsimd.memset(v_sb[jb][:, :, DH:DH + 1], 1.0)

            def emit_v(jb):
                ps = pp2.tile([P, DG], F32, tag="pp2", name="vps")
                for kb in range(KC):
                    nc.tensor.matmul(
                        ps,
                        lhsT=ctxv_sb[:, kb, jb * P:(jb + 1) * P],
                        rhs=wv_sb[:, kb, :],
                        start=(kb == 0), stop=(kb == KC - 1),
                    )
                nc.vector.tensor_copy(
                    v_sb[jb][:, :, 0:DH],
                    ps.rearrange("p (h d) -> p h d", h=HPC),
                )

            qt_sb = [[consts.tile([P, 512], BF16, tag=f"qt{t}{ich}",
                                  name=f"qt{t}{ich}") for ich in range(ICN)]
                     for t in range(TD)]

            def emit_qt(ich, t):
                ps = pp2.tile([P, 512], F32, tag="pp2", name="qtps")
                for pr in range(KQ // 2):
                    nc.tensor.matmul(
                        ps,
                        lhsT=wq_sb[:, 2 * pr:2 * pr + 2, t * P:(t + 1) * P],
                        rhs=xq_sb[:, ich, 2 * pr:2 * pr + 2, :],
                        start=(pr == 0), stop=(pr == KQ // 2 - 1),
                        perf_mode=DR,
                    )
                nc.vector.tensor_copy(qt_sb[t][ich], ps)

            emit_qt(0, 0)
            for w in range(8):
                nc.tensor.matmul(jps[:, :P], lhsT=junk, rhs=junk,
                                 start=True, stop=True)
            emit_qt(1, 0)
            emit_kt(0)

            o2t_sb = [[consts.tile([P, IH], BF16, tag=f"o2t{half}{t}",
                                   name=f"o2t{half}{t}")
                       for t in range(TD)] for half in range(2)]

            avtile = [None]
            av1h = [None, None]

            def emit_av_par(hp, par, ets, jbs, av=None, avh=None):
                for jb in jbs:
                    for nch in range(2):
                        csl = slice(nch * 512, (nch + 1) * 512)
                        if avh is not None:
                            tgt = avh[nch][:, :]
                        else:
                            tgt = (av if av is not None
                                   else avtile[0])[:, csl]
                        nc.tensor.matmul(
                            tgt,
                            lhsT=v_sb[jb][:, 2 * hp + par, :],
                            rhs=ets[par][jb][:, csl],
                            start=(jb == 0), stop=(jb == JBN - 1),
                        )

            def emit_norm(half, hp, par, src=None, dma_eng=None):
                """Copy av->araw (decouple psum + DMA needs SBUF src),
                then transpose/recip/broadcast via DRAM, then multiply."""
                h = 2 * hp + par
                if dma_eng is None:
                    dma_eng = nc.gpsimd
                araw = misc.tile([DH + 1, IH], F32, tag="araw", name="araw")
                nc.vector.tensor_copy(araw, src if src is not None
                                      else avtile[0])
                bidx = half * HPC + h
                dma_eng.dma_start(out=brc[bidx:bidx + 1, :],
                                  in_=araw[DH:DH + 1, :])
                rcol = misc.tile([8, IH // 8], F32, tag="rcol", name="rcol")
                dma_eng.dma_start(
                    out=rcol,
                    in_=brc[bidx, :].rearrange("(p t) -> p t", p=8),
                )
                rrec = misc.tile([8, IH // 8], F32, tag="rrec", name="rrec")
                nc.vector.reciprocal(rrec, rcol)
                dma_eng.dma_start(
                    out=brc2[bidx, :].rearrange("(p t) -> p t", p=8),
                    in_=rrec,
                )
                bc = misc.tile([DH, IH], F32, tag="bc", name="bc")
                row = brc2[bidx:bidx + 1, :]
                dma_eng.dma_start(
                    out=bc,
                    in_=bass.AP(tensor=row.tensor, offset=row.offset,
                                ap=[[0, DH]] + row.ap[1:]),
                )
                nc.vector.tensor_mul(
                    o2t_sb[half][hp][par * DH:par * DH + DH, :],
                    araw[0:DH, :], bc
                )

            def norm_start(half, hp, par, src_av, dma_eng, src_halves=None):
                h = 2 * hp + par
                araw = misc.tile([DH + 1, IH], F32, tag="araw", name="araw")
                if src_halves is not None:
                    nc.vector.tensor_copy(araw[:, 0:512], src_halves[0])
                    nc.vector.tensor_copy(araw[:, 512:1024], src_halves[1])
                else:
                    nc.vector.tensor_copy(araw, src_av)
                bidx = half * HPC + h
                dma_eng.dma_start(out=brc[bidx:bidx + 1, :],
                                  in_=araw[DH:DH + 1, :])
                rcol = misc.tile([8, IH // 8], F32, tag="rcol", name="rcol")
                dma_eng.dma_start(
                    out=rcol,
                    in_=brc[bidx, :].rearrange("(p t) -> p t", p=8),
                )
                return araw, rcol, bidx

            def norm_recip(st, dma_eng):
                araw, rcol, bidx = st
                rrec = misc.tile([8, IH // 8], F32, tag="rrec", name="rrec")
                nc.vector.reciprocal(rrec, rcol)
                dma_eng.dma_start(
                    out=brc2[bidx, :].rearrange("(p t) -> p t", p=8),
                    in_=rrec,
                )
                bc = misc.tile([DH, IH], F32, tag="bc", name="bc")
                row = brc2[bidx:bidx + 1, :]
                dma_eng.dma_start(
                    out=bc,
                    in_=bass.AP(tensor=row.tensor, offset=row.offset,
                                ap=[[0, DH]] + row.ap[1:]),
                )
                return araw, bc

            def norm_mult(half, hp, par, st2):
                araw, bc = st2
                nc.vector.tensor_mul(
                    o2t_sb[half][hp][par * DH:par * DH + DH, :],
                    araw[0:DH, :], bc
                )

            def emit_wo_m(half, m, deep, act_evac=False):
                # psum evacuation on DVE (ScalarE only in the drain, where
                # exps are done); bf16 staging (host accumulates in fp32)
                r0 = half * IH + m * P
                ot = outp.tile([P, E], BF16, tag="ot", name="ot")
                if deep:
                    big = pp.tile([P, IH], F32, tag="pp", name="wobig")
                    pss = [big[:, 0:512], big[:, 512:1024]]
                else:
                    pss = [pp2.tile([P, 512], F32, tag="pp2",
                                    name=f"wopp{n}") for n in range(2)]
                for t in range(TD):
                    for nch in range(2):
                        nc.tensor.matmul(
                            pss[nch],
                            lhsT=o2t_sb[half][t][:, m * P:(m + 1) * P],
                            rhs=wo_sb[:, t, nch * 512:(nch + 1) * 512],
                            start=(t == 0), stop=(t == TD - 1),
                        )
                for nch in range(2):
                    dst = ot[:, nch * 512:(nch + 1) * 512]
                    if nch == 1 and (deep or act_evac):
                        nc.scalar.activation(
                            out=dst, in_=pss[nch],
                            func=mybir.ActivationFunctionType.Copy)
                    else:
                        nc.vector.tensor_copy(dst, pss[nch])
                nc.sync.dma_start(out=out[r0:r0 + P, :], in_=ot)

            # ---- attention schedule: per-jb fine interleave so the
            # in-order PE stream never bursts long enough to starve ACT.
            pending = None
            for k, (half, hp) in enumerate([(0, 0), (0, 1), (1, 0), (1, 1)]):
                extras = []
                if k == 0:
                    extras = ([lambda: emit_qt(0, 1),
                               lambda: emit_kt(1), lambda: emit_qt(1, 1)]
                              + [(lambda jb=jb: emit_v(jb))
                                 for jb in range(JBN)]
                              + [lambda: emit_qt(2, 0), lambda: emit_qt(3, 0)])
                elif k == 1:
                    extras = [lambda: emit_qt(2, 1), lambda: emit_qt(3, 1)]
                elif k == 3:
                    extras = []
                prev = pending
                if prev is not None:
                    avtile[0] = avpool.tile([DH + 1, IH], F32, tag="av",
                                            name="av")
                avq = []
                if prev is not None:
                    avq = ([(0, jb) for jb in range(JBN)] + ["norm0"]
                           + [(1, jb) for jb in range(JBN)] + ["norm1"])
                if k == 3:
                    # pre-accumulate the drain group's par0 AV (jb 0-5)
                    # into two pp2 bank-halves while exps still run
                    av1h[0] = pp2.tile([DH + 1, 512], F32, tag="pp2",
                                       name="av1h0")
                    av1h[1] = pp2.tile([DH + 1, 512], F32, tag="pp2",
                                       name="av1h1")
                    avq += [("d1", jb) for jb in range(6)]

                def pop_av():
                    item = avq.pop(0)
                    if item == "norm0":
                        emit_norm(prev[0], prev[1], 0)
                        avtile[0] = avpool.tile([DH + 1, IH], F32, tag="av",
                                                name="av")
                    elif item == "norm1":
                        emit_norm(prev[0], prev[1], 1)
                    elif item[0] == "d1":
                        emit_av_par(hp, 0, ets, [item[1]], avh=av1h)
                    else:
                        emit_av_par(prev[1], item[0], prev[2], [item[1]])

                t = hp
                ets = [[None] * JBN, [None] * JBN]
                for jb in range(JBN):
                    scs = []
                    for par in range(2):
                        prow = par * DH
                        sc = pp.tile([P, IH], F32, tag="pp", name=f"sc{par}")
                        for nch in range(2):
                            nc.tensor.matmul(
                                sc[:, nch * 512:(nch + 1) * 512],
                                lhsT=kt_sb[t][prow:prow + DH,
                                              jb * P:(jb + 1) * P],
                                rhs=qt_sb[t][half * 2 + nch][prow:prow + DH, :],
                                start=True, stop=True,
                            )
                        scs.append(sc)
                    for par in range(2):
                        et = expp.tile([P, IH], BF16, tag="et",
                                       name=f"et{par}")
                        nc.scalar.activation(
                            out=et, in_=scs[par],
                            func=mybir.ActivationFunctionType.Exp,
                            scale=0.125 / (WSCALE * WSCALE),
                        )
                        ets[par][jb] = et
                    for _ in range(3):
                        if avq:
                            pop_av()
                    for _ in range(2):
                        if extras:
                            extras.pop(0)()
                while avq:
                    pop_av()
                while extras:
                    extras.pop(0)()
                pending = (half, hp, ets)

            # ---- drain the last pair: par0 AV burst -> its norm chain
            # launches (sync queue) while par1 AV + Wo(0) m4-7 keep the PE
            # busy; par1 norm on the gpsimd queue runs concurrently; junk
            # keeps the HAM awake until Wo(half1).
            half, hp, ets = pending
            # par1 into the avpool slot (freed by (1,0)'s norm1 araw copy),
            # emitted in dependency-time order against par0's last units
            av2 = avpool.tile([DH + 1, IH], F32, tag="av", name="av2")
            emit_av_par(hp, 1, ets, range(6), av=av2)
            emit_av_par(hp, 0, ets, [6], avh=av1h)
            emit_av_par(hp, 1, ets, [6], av=av2)
            emit_av_par(hp, 0, ets, [7], avh=av1h)
            emit_av_par(hp, 1, ets, [7], av=av2)
            st0 = norm_start(half, hp, 0, None, nc.gpsimd, src_halves=av1h)
            st1 = norm_start(half, hp, 1, av2, nc.sync)
            st0b = norm_recip(st0, nc.sync)
            st1b = norm_recip(st1, nc.sync)
            norm_mult(half, hp, 0, st0b)
            norm_mult(half, hp, 1, st1b)
            for m in range(8):
                emit_wo_m(0, m, True, act_evac=True)
            jps2 = pp.tile([P, IH], F32, tag="pp", name="jps2")
            for w in range(20):
                nc.tensor.matmul(jps2[:, 0:512], lhsT=junk, rhs=qt_sb[0][0],
                                 start=True, stop=True)
            for m in range(8):
                emit_wo_m(1, m, True)

    nc.compile()
    return nc


_NC_CACHE = None


def _get_nc():
    global _NC_CACHE
    if _NC_CACHE is None:
        _NC_CACHE = _build()
    return _NC_CACHE


def _sbuf_image(a, dtype):
    """[KB*128, R] row-major -> [128, KB*R]: partition p holds the
    concatenation of rows {kb*128+p} (one contiguous run per partition)."""
    kb = a.shape[0] // P
    return np.ascontiguousarray(
        a.reshape(kb, P, a.shape[1]).transpose(1, 0, 2).reshape(P, -1)
    ).astype(dtype)


def _x_image(xtb):
    """x^T [1024, 2048] -> per partition: [ich, kb, 512] contiguous."""
    r = xtb.reshape(KQ, P, ICN, 512).transpose(1, 2, 0, 3)
    return np.ascontiguousarray(r.reshape(P, -1)).astype(
        ml_dtypes.float8_e4m3)


def _make_in_maps(x, context, Wq, Wk, Wv, Wo):
    FP8NP = ml_dtypes.float8_e4m3
    in_maps = []
    for c in range(N_CORES):
        b, hg = c // 2, c % 2
        sl = slice(hg * DG, (hg + 1) * DG)
        in_maps.append({
            "xt": _x_image(x[b].T),
            "ctxt": _sbuf_image(context[b].T, FP8NP),
            "wq": _sbuf_image(Wq[:, sl] * WSCALE, FP8NP),
            "wk": _sbuf_image(Wk[:, sl] * WSCALE, FP8NP),
            "wv": _sbuf_image(Wv[:, sl], ml_dtypes.bfloat16),
            "ctxv": _sbuf_image(context[b].T, ml_dtypes.bfloat16),
            "wo": _sbuf_image(Wo[sl, :], ml_dtypes.bfloat16),
        })
    return in_maps


def _run(inputs, trace=False):
    x = np.asarray(inputs["x"], dtype=np.float32)
    context = np.asarray(inputs["context"], dtype=np.float32)
    Wq = np.asarray(inputs["Wq"], dtype=np.float32)
    Wk = np.asarray(inputs["Wk"], dtype=np.float32)
    Wv = np.asarray(inputs["Wv"], dtype=np.float32)
    Wo = np.asarray(inputs["Wo"], dtype=np.float32)
    bo = np.asarray(inputs["bo"], dtype=np.float32)

    res = run_bass_kernel_spmd(
        _get_nc(), _make_in_maps(x, context, Wq, Wk, Wv, Wo),
        core_ids=list(range(N_CORES)), trace=trace,
    )
    parts = [np.asarray(r["out"], dtype=np.float32) for r in res.results]
    outv = np.stack([parts[2 * b] + parts[2 * b + 1] + bo for b in range(B)])
    return outv.astype(np.float32), res


def kernel(**inputs) -> np.ndarray:
    outv, _ = _run(inputs, trace=False)
    return outv


# revision 18
# speedup vs baseline: 1.0128x; 1.0128x over previous
"""Cross-attention kernel for 8 TRN2 NeuronCores (Bass/Tile).

Reference (fp32):
    q = x @ Wq; k = ctx @ Wk; v = ctx @ Wv        (8 heads, d=64)
    sim = q k^T * d^-0.5 ; attn = softmax(sim)
    out = (attn v) @ Wo + bo

Sharding (data-parallel, no FLOP duplication): core c -> batch c//2,
head-group c%2 (4 heads).  Each core computes a partial [2048, 1024]
output; the host sums the two partials per batch and adds bo.

Per-core dataflow:
  - inputs x/ctx and Wq/Wk/Wv ship as fp8e4 (weights pre-scaled x32 on
    the host; fp8 noise enters PRE-softmax where the 1/8 sim scaling
    damps it to ~1e-3, unlike post-softmax fp8 which has no sqrt(N)
    attenuation) -- halves input DMA and doubles projection matmuls
    via DoubleRow (two K-tiles per instruction)
  - QT[d,i] = Wq^T x^T ; KT[d,j] = Wk^T ctx^T ; V[j,d] = ctx Wv, all
    evacuated to bf16; V gets a ones column ([V_h | 1], memset)
  - simT[j,i] = KT_h @ QT_h (K=64 bf16; head pairs on PE row groups),
    values x1024 from the weight scaling
  - expT = exp(sim/(8*1024)) on ScalarE -> bf16 (unsafe softmax)
  - av = [V_h|1]^T @ expT (bf16) -> [65, i] fp32 PSUM; row 64 = denom
  - norm: denom row -> DRAM -> [8,128] transpose read -> reciprocal
    (x 1/32 undoes the Wv scale) -> DRAM -> 0-stride broadcast read ->
    multiply -> o2t bf16
  - out = o2tT^T @ Wo (bf16) -> psum -> bf16 staging -> DRAM; host
    accumulates partials in fp32 and adds bo
The kernel is ScalarE-bound (64 exp ops over 8.4M elements, ~71us);
PE work (~70us) is balanced against it so the HAM clock never sees an
idle Tensor engine mid-schedule.
"""

import numpy as np
import ml_dtypes

import concourse.bass as bass
import concourse.tile as tile
from concourse import bacc, mybir
from concourse.bass_utils import run_bass_kernel_spmd

B = 4
I = 2048
J = 1024
FQ = 1024
FC = 768
DH = 64
HPC = 4
DG = HPC * DH      # 256
E = 1024
P = 128
N_CORES = 8
IH = I // 2        # 1024
WSCALE = 32.0      # host premultiplier on Wq/Wk/Wv (fp8 subnormal dodge)

F32 = mybir.dt.float32
BF16 = mybir.dt.bfloat16
FP8 = mybir.dt.float8e4

KQ = FQ // P       # 8
KC = FC // P       # 6
TD = DG // P       # 2
JBN = J // P       # 8
ICN = 4            # x i-chunks
KCA = 2            # ctx kb's in first half (pair-aligned)
DR = mybir.MatmulPerfMode.DoubleRow


def _build():
    nc = bacc.Bacc()
    xt = nc.declare_dram_parameter("xt", [P, KQ * I], FP8, isOutput=False)
    ctxt = nc.declare_dram_parameter("ctxt", [P, KC * J], FP8, isOutput=False)
    wq = nc.declare_dram_parameter("wq", [P, KQ * DG], FP8, isOutput=False)
    wk = nc.declare_dram_parameter("wk", [P, KC * DG], FP8, isOutput=False)
    wv = nc.declare_dram_parameter("wv", [P, KC * DG], BF16, isOutput=False)
    ctxv = nc.declare_dram_parameter("ctxv", [P, KC * J], BF16,
                                     isOutput=False)
    wo = nc.declare_dram_parameter("wo", [P, TD * E], BF16, isOutput=False)
    out = nc.declare_dram_parameter("out", [I, E], BF16, isOutput=True)
    brc = nc.dram_tensor("brc", [2 * HPC, IH], F32)
    brc2 = nc.dram_tensor("brc2", [2 * HPC, IH], F32)

    with tile.TileContext(nc) as tc:
        with (
            tc.tile_pool(name="consts", bufs=1) as consts,
            tc.tile_pool(name="expp", bufs=36) as expp,
            tc.tile_pool(name="misc", bufs=3) as misc,
            tc.tile_pool(name="outp", bufs=6) as outp,
            tc.tile_pool(name="pp", bufs=2, space="PSUM") as pp,
            tc.tile_pool(name="pp2", bufs=2, space="PSUM") as pp2,
            tc.tile_pool(name="avp", bufs=1, space="PSUM") as avpool,
        ):
            # ---- PE warm-up: junk matmuls trip the HAM clock-gate (cold
            # PE runs 0.65-1.2 GHz) and bridge until x0 lands
            junk = consts.tile([P, P], BF16, tag="junk")
            nc.vector.memset(junk, 0.0)
            jps = pp2.tile([P, 512], F32, tag="pp2", name="jps")
            for w in range(34):
                nc.tensor.matmul(jps[:, :P], lhsT=junk, rhs=junk,
                                 start=True, stop=True)

            # ---- loads, ordered by the critical path to the first exp:
            # wq+x0 (QT is the PE's first real work), x1, ctx halves + wk
            # (KT), then the rest.  All fp8 -> half the bytes.
            wq_sb = consts.tile([P, KQ, DG], FP8, tag="wq_sb")
            nc.sync.dma_start(
                out=wq_sb, in_=wq[:, :].rearrange("p (kb d) -> p kb d", kb=KQ))
            xq_sb = consts.tile([P, ICN, KQ, 512], FP8, tag="xq_sb")

            def load_x(ich):
                nc.sync.dma_start(
                    out=xq_sb[:, ich],
                    in_=xt[:, ich * KQ * 512:(ich + 1) * KQ * 512]
                    .rearrange("p (kb i) -> p kb i", kb=KQ))

            load_x(0)
            load_x(1)
            # ctx split (2,4) kb's: DoubleRow kb-pairs never straddle tiles
            ctx_sb = [consts.tile([P, KCA, J], FP8, tag="ctxA", name="ctxA"),
                      consts.tile([P, KC - KCA, J], FP8, tag="ctxB",
                                  name="ctxB")]
            nc.sync.dma_start(
                out=ctx_sb[0],
                in_=ctxt[:, :KCA * J].rearrange("p (kb j) -> p kb j", kb=KCA))
            wk_sb = consts.tile([P, KC, DG], FP8, tag="wk_sb")
            nc.sync.dma_start(
                out=wk_sb, in_=wk[:, :].rearrange("p (kb d) -> p kb d", kb=KC))
            nc.sync.dma_start(
                out=ctx_sb[1],
                in_=ctxt[:, KCA * J:].rearrange("p (kb j) -> p kb j",
                                                kb=KC - KCA))
            wv_sb = consts.tile([P, KC, DG], BF16, tag="wv_sb")
            nc.sync.dma_start(
                out=wv_sb, in_=wv[:, :].rearrange("p (kb d) -> p kb d", kb=KC))
            ctxv_sb = consts.tile([P, KC, J], BF16, tag="ctxv_sb")
            nc.sync.dma_start(
                out=ctxv_sb,
                in_=ctxv[:, :].rearrange("p (kb j) -> p kb j", kb=KC))
            load_x(2)
            load_x(3)
            wo_sb = consts.tile([P, TD, E], BF16, tag="wo_sb")
            nc.sync.dma_start(
                out=wo_sb, in_=wo[:, :].rearrange("p (kb e) -> p kb e", kb=TD))

            # kb-pair views for DoubleRow: (tile, local pair index)
            def ctx_pair(pr):
                return ctx_sb[0] if pr == 0 else ctx_sb[1], 0 if pr == 0 \
                    else pr - 1

            # ---- projections (fp8 DoubleRow: 2 kb per matmul)
            kt_sb = [consts.tile([P, J], BF16, tag=f"kt{t}", name=f"kt{t}")
                     for t in range(TD)]

            def emit_kt(t):
                for nch in range(2):
                    ps = pp2.tile([P, 512], F32, tag="pp2", name="ktps")
                    for pr in range(KC // 2):
                        cs, lp = ctx_pair(pr)
                        nc.tensor.matmul(
                            ps,
                            lhsT=wk_sb[:, 2 * pr:2 * pr + 2,
                                       t * P:(t + 1) * P],
                            rhs=cs[:, 2 * lp:2 * lp + 2,
                                   nch * 512:(nch + 1) * 512],
                            start=(pr == 0), stop=(pr == KC // 2 - 1),
                            perf_mode=DR,
                        )
                    nc.vector.tensor_copy(
                        kt_sb[t][:, nch * 512:(nch + 1) * 512], ps)

            v_sb = [consts.tile([P, HPC, DH + 1], BF16, tag=f"v{jb}",
                                name=f"v{jb}") for jb in range(JBN)]
            for jb in range(JBN):
                nc.gpsimd.memset(v_sb[jb][:, :, DH:DH + 1], 1.0)

            def emit_v(jb):
                ps = pp2.tile([P, DG], F32, tag="pp2", name="vps")
                for kb in range(KC):
                    nc.tensor.matmul(
                        ps,
                        lhsT=ctxv_sb[:, kb, jb * P:(jb + 1) * P],
                        rhs=wv_sb[:, kb, :],
                        start=(kb == 0), stop=(kb == KC - 1),
                    )
                nc.vector.tensor_copy(
                    v_sb[jb][:, :, 0:DH],
                    ps.rearrange("p (h d) -> p h d", h=HPC),
                )

            qt_sb = [[consts.tile([P, 512], BF16, tag=f"qt{t}{ich}",
                                  name=f"qt{t}{ich}") for ich in range(ICN)]
                     for t in range(TD)]

            def emit_qt(ich, t):
                ps = pp2.tile([P, 512], F32, tag="pp2", name="qtps")
                for pr in range(KQ // 2):
                    nc.tensor.matmul(
                        ps,
                        lhsT=wq_sb[:, 2 * pr:2 * pr + 2, t * P:(t + 1) * P],
                        rhs=xq_sb[:, ich, 2 * pr:2 * pr + 2, :],
                        start=(pr == 0), stop=(pr == KQ // 2 - 1),
                        perf_mode=DR,
                    )
                nc.vector.tensor_copy(qt_sb[t][ich], ps)

            emit_qt(0, 0)
            for w in range(8):
                nc.tensor.matmul(jps[:, :P], lhsT=junk, rhs=junk,
                                 start=True, stop=True)
            emit_qt(1, 0)
            emit_kt(0)

            o2t_sb = [[consts.tile([P, IH], BF16, tag=f"o2t{half}{t}",
                                   name=f"o2t{half}{t}")
                       for t in range(TD)] for half in range(2)]

            avtile = [None]
            av1h = [None, None]

            def emit_av_par(hp, par, ets, jbs, av=None, avh=None):
                for jb in jbs:
                    for nch in range(2):
                        csl = slice(nch * 512, (nch + 1) * 512)
                        if avh is not None:
                            tgt = avh[nch][:, :]
                        else:
                            tgt = (av if av is not None
                                   else avtile[0])[:, csl]
                        nc.tensor.matmul(
                            tgt,
                            lhsT=v_sb[jb][:, 2 * hp + par, :],
                            rhs=ets[par][jb][:, csl],
                            start=(jb == 0), stop=(jb == JBN - 1),
                        )

            def emit_norm(half, hp, par, src=None, dma_eng=None):
                """Copy av->araw (decouple psum + DMA needs SBUF src),
                then transpose/recip/broadcast via DRAM, then multiply."""
                h = 2 * hp + par
                if dma_eng is None:
                    dma_eng = nc.gpsimd
                araw = misc.tile([DH + 1, IH], F32, tag="araw", name="araw")
                nc.vector.tensor_copy(araw, src if src is not None
                                      else avtile[0])
                bidx = half * HPC + h
                dma_eng.dma_start(out=brc[bidx:bidx + 1, :],
                                  in_=araw[DH:DH + 1, :])
                rcol = misc.tile([8, IH // 8], F32, tag="rcol", name="rcol")
                dma_eng.dma_start(
                    out=rcol,
                    in_=brc[bidx, :].rearrange("(p t) -> p t", p=8),
                )
                rrec = misc.tile([8, IH // 8], F32, tag="rrec", name="rrec")
                nc.vector.reciprocal(rrec, rcol)
                dma_eng.dma_start(
                    out=brc2[bidx, :].rearrange("(p t) -> p t", p=8),
                    in_=rrec,
                )
                bc = misc.tile([DH, IH], F32, tag="bc", name="bc")
                row = brc2[bidx:bidx + 1, :]
                dma_eng.dma_start(
                    out=bc,
                    in_=bass.AP(tensor=row.tensor, offset=row.offset,
                                ap=[[0, DH]] + row.ap[1:]),
                )
                nc.vector.tensor_mul(
                    o2t_sb[half][hp][par * DH:par * DH + DH, :],
                    araw[0:DH, :], bc
                )

            def norm_start(half, hp, par, src_av, dma_eng, src_halves=None):
                h = 2 * hp + par
                araw = misc.tile([DH + 1, IH], F32, tag="araw", name="araw")
                if src_halves is not None:
                    nc.vector.tensor_copy(araw[:, 0:512], src_halves[0])
                    nc.vector.tensor_copy(araw[:, 512:1024], src_halves[1])
                else:
                    nc.vector.tensor_copy(araw, src_av)
                bidx = half * HPC + h
                dma_eng.dma_start(out=brc[bidx:bidx + 1, :],
                                  in_=araw[DH:DH + 1, :])
                rcol = misc.tile([8, IH // 8], F32, tag="rcol", name="rcol")
                dma_eng.dma_start(
                    out=rcol,
                    in_=brc[bidx, :].rearrange("(p t) -> p t", p=8),
                )
                return araw, rcol, bidx

            def norm_recip(st, dma_eng):
                araw, rcol, bidx = st
                rrec = misc.tile([8, IH // 8], F32, tag="rrec", name="rrec")
                nc.vector.reciprocal(rrec, rcol)
                dma_eng.dma_start(
                    out=brc2[bidx, :].rearrange("(p t) -> p t", p=8),
                    in_=rrec,
                )
                bc = misc.tile([DH, IH], F32, tag="bc", name="bc")
                row = brc2[bidx:bidx + 1, :]
                dma_eng.dma_start(
                    out=bc,
                    in_=bass.AP(tensor=row.tensor, offset=row.offset,
                                ap=[[0, DH]] + row.ap[1:]),
                )
                return araw, bc

            def norm_mult(half, hp, par, st2):
                araw, bc = st2
                nc.vector.tensor_mul(
                    o2t_sb[half][hp][par * DH:par * DH + DH, :],
                    araw[0:DH, :], bc
                )

            def emit_wo_m(half, m, deep, act_evac=False):
                # psum evacuation on DVE (ScalarE only in the drain, where
                # exps are done); bf16 staging (host accumulates in fp32)
                r0 = half * IH + m * P
                ot = outp.tile([P, E], BF16, tag="ot", name="ot")
                if deep:
                    big = pp.tile([P, IH], F32, tag="pp", name="wobig")
                    pss = [big[:, 0:512], big[:, 512:1024]]
                else:
                    pss = [pp2.tile([P, 512], F32, tag="pp2",
                                    name=f"wopp{n}") for n in range(2)]
                for t in range(TD):
                    for nch in range(2):
                        nc.tensor.matmul(
                            pss[nch],
                            lhsT=o2t_sb[half][t][:, m * P:(m + 1) * P],
                            rhs=wo_sb[:, t, nch * 512:(nch + 1) * 512],
                            start=(t == 0), stop=(t == TD - 1),
                        )
                for nch in range(2):
                    dst = ot[:, nch * 512:(nch + 1) * 512]
                    if nch == 1 and (deep or act_evac):
                        nc.scalar.activation(
                            out=dst, in_=pss[nch],
                            func=mybir.ActivationFunctionType.Copy)
                    else:
                        nc.vector.tensor_copy(dst, pss[nch])
                # out stores ride the gpsimd queue: the sync queue must
                # stay short for the drain's norm-chain hops
                nc.gpsimd.dma_start(out=out[r0:r0 + P, :], in_=ot)

            # ---- attention schedule: per-jb fine interleave so the
            # in-order PE stream never bursts long enough to starve ACT.
            pending = None
            for k, (half, hp) in enumerate([(0, 0), (0, 1), (1, 0), (1, 1)]):
                extras = []
                if k == 0:
                    extras = ([lambda: emit_qt(0, 1),
                               lambda: emit_kt(1), lambda: emit_qt(1, 1)]
                              + [(lambda jb=jb: emit_v(jb))
                                 for jb in range(JBN)]
                              + [lambda: emit_qt(2, 0), lambda: emit_qt(3, 0)])
                elif k == 1:
                    extras = [lambda: emit_qt(2, 1), lambda: emit_qt(3, 1)]
                elif k == 3:
                    extras = []
                prev = pending
                if prev is not None:
                    avtile[0] = avpool.tile([DH + 1, IH], F32, tag="av",
                                            name="av")
                avq = []
                if prev is not None:
                    avq = ([(0, jb) for jb in range(JBN)] + ["norm0"]
                           + [(1, jb) for jb in range(JBN)] + ["norm1"])
                if k == 3:
                    # pre-accumulate the drain group's par0 AV (jb 0-5)
                    # into two pp2 bank-halves while exps still run
                    av1h[0] = pp2.tile([DH + 1, 512], F32, tag="pp2",
                                       name="av1h0")
                    av1h[1] = pp2.tile([DH + 1, 512], F32, tag="pp2",
                                       name="av1h1")
                    avq += [("d1", jb) for jb in range(6)]

                def pop_av():
                    item = avq.pop(0)
                    if item == "norm0":
                        emit_norm(prev[0], prev[1], 0)
                        avtile[0] = avpool.tile([DH + 1, IH], F32, tag="av",
                                                name="av")
                    elif item == "norm1":
                        emit_norm(prev[0], prev[1], 1)
                    elif item[0] == "d1":
                        emit_av_par(hp, 0, ets, [item[1]], avh=av1h)
                    else:
                        emit_av_par(prev[1], item[0], prev[2], [item[1]])

                t = hp
                ets = [[None] * JBN, [None] * JBN]
                for jb in range(JBN):
                    scs = []
                    for par in range(2):
                        prow = par * DH
                        sc = pp.tile([P, IH], F32, tag="pp", name=f"sc{par}")
                        for nch in range(2):
                            nc.tensor.matmul(
                                sc[:, nch * 512:(nch + 1) * 512],
                                lhsT=kt_sb[t][prow:prow + DH,
                                              jb * P:(jb + 1) * P],
                                rhs=qt_sb[t][half * 2 + nch][prow:prow + DH, :],
                                start=True, stop=True,
                            )
                        scs.append(sc)
                    for par in range(2):
                        et = expp.tile([P, IH], BF16, tag="et",
                                       name=f"et{par}")
                        nc.scalar.activation(
                            out=et, in_=scs[par],
                            func=mybir.ActivationFunctionType.Exp,
                            scale=0.125 / (WSCALE * WSCALE),
                        )
                        ets[par][jb] = et
                    for _ in range(3):
                        if avq:
                            pop_av()
                    for _ in range(2):
                        if extras:
                            extras.pop(0)()
                while avq:
                    pop_av()
                while extras:
                    extras.pop(0)()
                pending = (half, hp, ets)

            # ---- drain the last pair: par0 AV burst -> its norm chain
            # launches (sync queue) while par1 AV + Wo(0) m4-7 keep the PE
            # busy; par1 norm on the gpsimd queue runs concurrently; junk
            # keeps the HAM awake until Wo(half1).
            half, hp, ets = pending
            # par1 into the avpool slot (freed by (1,0)'s norm1 araw copy),
            # emitted in dependency-time order against par0's last units
            av2 = avpool.tile([DH + 1, IH], F32, tag="av", name="av2")
            emit_av_par(hp, 1, ets, range(6), av=av2)
            emit_av_par(hp, 0, ets, [6], avh=av1h)
            emit_av_par(hp, 1, ets, [6], av=av2)
            emit_av_par(hp, 0, ets, [7], avh=av1h)
            emit_av_par(hp, 1, ets, [7], av=av2)
            st0 = norm_start(half, hp, 0, None, nc.gpsimd, src_halves=av1h)
            st1 = norm_start(half, hp, 1, av2, nc.sync)
            st0b = norm_recip(st0, nc.sync)
            st1b = norm_recip(st1, nc.sync)
            norm_mult(half, hp, 0, st0b)
            norm_mult(half, hp, 1, st1b)
            for m in range(8):
                emit_wo_m(0, m, True, act_evac=True)
            jps2 = pp.tile([P, IH], F32, tag="pp", name="jps2")
            for w in range(20):
                nc.tensor.matmul(jps2[:, 0:512], lhsT=junk, rhs=qt_sb[0][0],
                                 start=True, stop=True)
            for m in range(8):
                emit_wo_m(1, m, True)

    nc.compile()
    return nc


_NC_CACHE = None


def _get_nc():
    global _NC_CACHE
    if _NC_CACHE is None:
        _NC_CACHE = _build()
    return _NC_CACHE


def _sbuf_image(a, dtype):
    """[KB*128, R] row-major -> [128, KB*R]: partition p holds the
    concatenation of rows {kb*128+p} (one contiguous run per partition)."""
    kb = a.shape[0] // P
    return np.ascontiguousarray(
        a.reshape(kb, P, a.shape[1]).transpose(1, 0, 2).reshape(P, -1)
    ).astype(dtype)


def _x_image(xtb):
    """x^T [1024, 2048] -> per partition: [ich, kb, 512] contiguous."""
    r = xtb.reshape(KQ, P, ICN, 512).transpose(1, 2, 0, 3)
    return np.ascontiguousarray(r.reshape(P, -1)).astype(
        ml_dtypes.float8_e4m3)


def _make_in_maps(x, context, Wq, Wk, Wv, Wo):
    FP8NP = ml_dtypes.float8_e4m3
    in_maps = []
    for c in range(N_CORES):
        b, hg = c // 2, c % 2
        sl = slice(hg * DG, (hg + 1) * DG)
        in_maps.append({
            "xt": _x_image(x[b].T),
            "ctxt": _sbuf_image(context[b].T, FP8NP),
            "wq": _sbuf_image(Wq[:, sl] * WSCALE, FP8NP),
            "wk": _sbuf_image(Wk[:, sl] * WSCALE, FP8NP),
            "wv": _sbuf_image(Wv[:, sl], ml_dtypes.bfloat16),
            "ctxv": _sbuf_image(context[b].T, ml_dtypes.bfloat16),
            "wo": _sbuf_image(Wo[sl, :], ml_dtypes.bfloat16),
        })
    return in_maps


def _run(inputs, trace=False):
    x = np.asarray(inputs["x"], dtype=np.float32)
    context = np.asarray(inputs["context"], dtype=np.float32)
    Wq = np.asarray(inputs["Wq"], dtype=np.float32)
    Wk = np.asarray(inputs["Wk"], dtype=np.float32)
    Wv = np.asarray(inputs["Wv"], dtype=np.float32)
    Wo = np.asarray(inputs["Wo"], dtype=np.float32)
    bo = np.asarray(inputs["bo"], dtype=np.float32)

    res = run_bass_kernel_spmd(
        _get_nc(), _make_in_maps(x, context, Wq, Wk, Wv, Wo),
        core_ids=list(range(N_CORES)), trace=trace,
    )
    parts = [np.asarray(r["out"], dtype=np.float32) for r in res.results]
    outv = np.stack([parts[2 * b] + parts[2 * b + 1] + bo for b in range(B)])
    return outv.astype(np.float32), res


def kernel(**inputs) -> np.ndarray:
    outv, _ = _run(inputs, trace=False)
    return outv
